# revision 22
# baseline (speedup 1.0000x reference)
"""StyleGAN2-style modulated conv (per-sample 3x3, 256->256 ch, 128x128) on 8 TRN2 cores.

Data-parallel over batch: core c computes sample c entirely on-chip.

Host-side prep (cheap, 0.25% of FLOPs): style = w@A.T+b+1, demodulation, and
the (O,I,kh,kw) -> (I, ot*9*128 + kk*128 + o) transposed fp16 weight layout are
all folded into one per-sample weight tensor, and x is pre-padded to 130x130
fp16. The device kernel is then a pure shift-and-matmul conv: fp16 weights
(stationary, FWL) x fp16 activations (moving), fp32 PSUM accumulate.

Hybrid fp8 DoubleRow: tap (0,0) is computed for both input-channel halves in
a single fp8e4 DoubleRow matmul (virtual K=256, ~2x MACs in ~1.3x time),
replacing 2 of the 18 fp16 matmuls per accumulation chain. Error scales with
sqrt(fp8 fraction): f=1/9 gives 1.6e-2 max-rel-err (verified bit-exact on
host across all samples) against the 2e-2 gate; f=2/9 would exceed it.

Schedule notes:
- dummy warmup matmuls issue immediately (no input deps) so the HAM clock
  gate is at 8/8 before the first real matmul, which otherwise runs ~12 MMs
  at 1.2 GHz.
- startup DMAs split across both HWDGE rings (sync + scalar) to halve the
  serial ~0.6us-per-DMA issue cost on the critical path; group 0 is split by
  rows so the first sub-chains start before the whole tile lands.
  Steady-state x prefetch goes to the gpsimd SWDGE ring where a parked WAR
  wait can't head-of-line-block eviction/output traffic.
- PSUM is 8 independent single-bank tiles (not 2x4-bank): a fresh
  accumulation group never has a same-tile WAR against the previous
  sub-block's eviction read, which otherwise stalls the PE ~0.8us per chain
  once the sequencer run-ahead drains at the end of the kernel.
- y is written fp16 to a flat [CO, H*W] tensor so each 4-row store is one
  1024B-contiguous run per partition (256B runs would trigger sub-line-rate
  SDMA read-modify-write on HBM).
"""

import numpy as np
from contextlib import ExitStack

import concourse.bass as bass
import concourse.mybir as mybir
import concourse.tile as tile
from concourse import bacc

FP32 = mybir.dt.float32
FP16 = mybir.dt.float16
FP8 = mybir.dt.float8e4

B = 8
CI = 256
CO = 256
H = 128
W = 128
KS = 3
Z = 512
NKK = KS * KS          # 9 kernel taps
IT = CI // 128         # 2 input-channel tiles
OT = CO // 128         # 2 output-channel tiles
RG = 16                # output rows per group
G = H // RG            # 8 row groups
HP = H + 2             # padded height
WP = W + 2             # padded width
OBLK = NKK * 128       # 1152: weight columns per output-channel tile
NWARM = 18             # dummy matmuls to lift the HAM clock gate
EPS = 1e-8


def build_nc() -> bass.Bass:
    nc = bacc.Bacc("TRN2", target_bir_lowering=False, debug=False)
    xp_d = nc.dram_tensor("xp", [CI, HP, WP], FP16, kind="ExternalInput")
    wm_d = nc.dram_tensor("wm", [CI, OT * OBLK], FP16, kind="ExternalInput")
    xp8_d = nc.dram_tensor("xp8", [CI, HP, W], FP8, kind="ExternalInput")
    wdr_d = nc.dram_tensor("wdr", [128, IT, OT, 128], FP8, kind="ExternalInput")
    y_d = nc.dram_tensor("y", [CO, H * W], FP16, kind="ExternalOutput")

    with tile.TileContext(nc) as tc, ExitStack() as ctx:
        singles = ctx.enter_context(tc.tile_pool(name="singles", bufs=1))
        opool = ctx.enter_context(tc.tile_pool(name="og", bufs=4))
        cpsum = ctx.enter_context(tc.tile_pool(name="cpsum", bufs=8, space="PSUM"))

        # PE warmup: no input deps, so these dispatch the moment the engine
        # queues open and run while the first DMAs are still in flight.
        wz = singles.tile([128, 256], FP16, name="wz", tag="wz")
        nc.gpsimd.memset(wz, 0.0)
        warm = cpsum.tile([128, 512], FP32, name="warm", tag="pg")
        for _ in range(NWARM):
            nc.tensor.matmul(
                warm[:, 0:256], lhsT=wz[:, 0:128], rhs=wz, start=True, stop=True
            )

        wm_sb = [
            singles.tile([128, OT * OBLK], FP16, name=f"wm{it}", tag=f"wm{it}")
            for it in range(IT)
        ]
        xg = [
            [
                singles.tile(
                    [128, RG + 2, WP], FP16, name=f"xg{it}_{s}", tag=f"xg{it}_{s}"
                )
                for s in range(3)
            ]
            for it in range(IT)
        ]
        # fp8 copy of x for the DoubleRow tap (kk=0 -> dw=0, so only padded
        # cols 0..127 are needed and rows stay contiguous: 3D rhs AP with
        # j-step 18*128 % 16 == 0)
        xg8 = [
            singles.tile([128, IT, RG + 2, W], FP8, name=f"xg8_{s}", tag=f"xg8_{s}")
            for s in range(3)
        ]
        wdr_sb = singles.tile([128, IT, OT, 128], FP8, name="wdr", tag="wdr")
        nc.gpsimd.dma_start(out=wdr_sb, in_=wdr_d[:, :, :, :])

        def load_group(g: int, eng):
            for it in range(IT):
                eng.dma_start(
                    out=xg[it][g % 3],
                    in_=xp_d[it * 128:(it + 1) * 128, g * RG:g * RG + RG + 2, :],
                )
                eng.dma_start(
                    out=xg8[g % 3][:, it],
                    in_=xp8_d[it * 128:(it + 1) * 128, g * RG:g * RG + RG + 2, :],
                )

        # The first conv chain needs wm(ot0, it0) + xg(g0, it0) immediately and
        # the it1 halves ~2us later; spread those four DMAs across all three
        # DGE rings (sync/scalar HWDGE + gpsimd SWDGE) so their ~0.6us serial
        # issue slots overlap, and split g0 by rows so the first sub-chains
        # (rows 0..10) can start before the whole tile has landed.
        # first chain consumes wm taps in kk order: land taps 0..3 of it0/it1
        # first (96KB each) so the chain can start ~0.5us before the full
        # 295KB ot0 blocks would arrive
        TSPL = 4 * 128
        for it in range(IT):
            nc.sync.dma_start(
                out=wm_sb[it][:, 0:TSPL],
                in_=wm_d[it * 128:(it + 1) * 128, 0:TSPL],
            )
        for it in range(IT):
            nc.scalar.dma_start(
                out=xg[it][0][:, 0:11, :],
                in_=xp_d[it * 128:(it + 1) * 128, 0:11, :],
            )
        for it in range(IT):
            nc.sync.dma_start(
                out=wm_sb[it][:, TSPL:OBLK],
                in_=wm_d[it * 128:(it + 1) * 128, TSPL:OBLK],
            )
        for it in range(IT):
            nc.scalar.dma_start(
                out=xg8[0][:, it],
                in_=xp8_d[it * 128:(it + 1) * 128, 0:RG + 2, :],
            )
        for it in range(IT):
            nc.scalar.dma_start(
                out=xg[it][0][:, 11:RG + 2, :],
                in_=xp_d[it * 128:(it + 1) * 128, 11:RG + 2, :],
            )
        for it in range(IT):
            nc.sync.dma_start(
                out=wm_sb[it][:, OBLK:2 * OBLK],
                in_=wm_d[it * 128:(it + 1) * 128, OBLK:2 * OBLK],
            )
        load_group(1, nc.scalar)
        load_group(2, nc.sync)

        chain_no = [0]

        def conv_tile(g: int, ot: int):
            for sub in range(4):
                # The first two chains run tap 0 in fp16 (the fp16 weight
                # tensor has all 9 taps anyway): their fp8 DoubleRow tiles
                # can't clear the congested startup DMA rings in time, and a
                # full-fp16 chain costs only ~0.2us extra vs a ~1.1us stall.
                use_dr = chain_no[0] >= 2
                chain_no[0] += 1
                k0 = 1 if use_dr else 0
                pg = cpsum.tile([128, 512], FP32, name="pg", tag="pg")
                r = sub * 4  # first output row (group-local) of this 512-col block
                for it in range(IT):
                    for kk in range(k0, NKK):
                        dh, dw = kk // 3, kk % 3
                        nc.tensor.matmul(
                            pg,
                            lhsT=wm_sb[it][
                                :, ot * OBLK + kk * 128: ot * OBLK + (kk + 1) * 128
                            ],
                            rhs=xg[it][g % 3][:, r + dh:r + dh + 4, dw:dw + W],
                            start=(it == 0 and kk == k0),
                            stop=(not use_dr and it == IT - 1 and kk == NKK - 1),
                        )
                if use_dr:
                    # tap kk=0 for both input-channel halves in one fp8
                    # DoubleRow matmul (virtual K=256: ~2x MACs at the same
                    # 216ns as a regular N=512 matmul). Last in the chain so
                    # the fp16 part never waits on the fp8 tiles.
                    nc.tensor.matmul(
                        pg,
                        lhsT=wdr_sb[:, :, ot, :],
                        rhs=xg8[g % 3][:, :, r:r + 4, :],
                        start=False,
                        stop=True,
                        perf_mode=mybir.MatmulPerfMode.DoubleRow,
                    )
                og = opool.tile([128, 512], FP16, name="og", tag="og")
                if sub % 2 == 0:
                    nc.vector.tensor_copy(out=og, in_=pg)
                else:
                    nc.scalar.copy(out=og, in_=pg)
                r0 = g * RG + sub * 4
                nc.sync.dma_start(
                    out=y_d[ot * 128:(ot + 1) * 128, r0 * W:(r0 + 4) * W],
                    in_=og,
                )

        for g in range(G):
            if 1 <= g and g + 2 < G:
                load_group(g + 2, nc.gpsimd)
            for ot in range(OT):
                conv_tile(g, ot)
    nc.finalize()
    return nc


_CACHE: dict = {}


def _get_nc() -> bass.Bass:
    if "nc" not in _CACHE:
        _CACHE["nc"] = build_nc()
    return _CACHE["nc"]


def make_in_maps(x, w, weight, affine_w, affine_b):
    x = np.ascontiguousarray(x, dtype=np.float32)
    w = np.ascontiguousarray(w, dtype=np.float32)
    weight = np.ascontiguousarray(weight, dtype=np.float32)
    affine_w = np.ascontiguousarray(affine_w, dtype=np.float32)
    affine_b = np.ascontiguousarray(affine_b, dtype=np.float32)

    style = w @ affine_w.T + affine_b + 1.0                      # [b, I]
    wgt = weight[None] * style[:, None, :, None, None]           # [b, O, I, 3, 3]
    denom = 1.0 / np.sqrt((wgt * wgt).sum(axis=(2, 3, 4)) + EPS)  # [b, O]
    wmod = wgt * denom[:, :, None, None, None]                   # [b, O, I, 3, 3]
    # -> [b, i, ot, kk, ol]: lhsT slice [i, o] per (ot, kk) is contiguous
    wm = wmod.reshape(B, OT, 128, CI, NKK).transpose(0, 3, 1, 4, 2)
    wm = np.ascontiguousarray(wm).reshape(B, CI, OT * OBLK).astype(np.float16)

    xp = np.zeros((B, CI, HP, WP), np.float16)
    xp[:, :, 1:H + 1, 1:W + 1] = x

    from ml_dtypes import float8_e4m3
    # fp8 copy for the DoubleRow tap (kk=0): padded cols 0..127 only
    xp8 = np.zeros((B, CI, HP, W), float8_e4m3)
    xp8[:, :, 1:H + 1, 1:W] = x[:, :, :, 0:W - 1].astype(float8_e4m3)
    # wdr[i, j, ot, o] = wmod[ot*128+o, j*128+i, 0, 0] in fp8
    w8 = wmod[:, :, :, 0, 0].astype(float8_e4m3)          # [b, O, I]
    wdr = np.ascontiguousarray(
        w8.reshape(B, OT, 128, IT, 128).transpose(0, 4, 3, 1, 2)
    )
    return [
        {"xp": xp[c], "wm": wm[c], "xp8": xp8[c], "wdr": wdr[c]} for c in range(B)
    ]


def run_on_hw(inputs: dict, trace: bool = False, tmpdir: str | None = None):
    from concourse.bass_utils import run_bass_kernel_spmd

    nc = _get_nc()
    in_maps = make_in_maps(**inputs)
    res = run_bass_kernel_spmd(
        nc, in_maps, core_ids=list(range(B)), trace=trace, tmpdir=tmpdir
    )
    y = np.stack([r["y"] for r in res.results], axis=0)
    y = y.reshape(B, CO, H, W).astype(np.float32)
    return y, res


def kernel(x, w, weight, affine_w, affine_b):
    y, _ = run_on_hw(
        dict(x=x, w=w, weight=weight, affine_w=affine_w, affine_b=affine_b)
    )
    return y
